# revision 13
# baseline (speedup 1.0000x reference)
"""CCLoss (Pearson correlation loss) Trainium2 kernel, 8-way data parallel.

Problem: y_pred ~ (64,1,480,640) f32, y_true ~ (64,1,480,640) f32.
reference: per-sample z-score (ddof=1) over (1,480,640), r = corr-like ratio,
loss = -mean(r).

Strategy: shard batch (64) across 8 cores, 8 samples/core. Inputs are
converted to bf16 on the host (quantization perturbs the loss by ~1e-3
relative, far under the 2e-2 gate) which halves HBM traffic; the kernel is
memory-bound (~9.8MB/core at ~420GB/s = 23.5us stream floor), but the
per-sample moment reductions make compute the slightly-longer pole
(~3.7us/sample across three engines vs 2.93us of DMA).

Five per-sample sums, one pass over the data, balanced across engines
(measured bf16 rates: DVE stt 1.07ns/col, DVE tensor_tensor 0.55, ACT square
0.91 + 185ns/accum-read, PE colsum-matmul ~0.73 incl ldweights):
  - VectorE (DVE): sum(x*y) full via scalar_tensor_tensor accum;
                   sum(x*x) cols [XP:XP+XD) via stt; x*x product tile for
                   cols [0:XP) via 2x tensor_tensor (PE reduces it)
  - ScalarE (ACT): sum(y^2) full + sum(x*x) cols [XP+XD:F) via Square accum
  - TensorE (PE):  sum(x), sum(y), and colsums of the x*x product tile:
                   ones-one-hot-stationary matmuls (sample s uses a [128,8]
                   stationary all-ones in column s) accumulate per-sample-row
                   column sums in PSUM; ACT copy+accum reduces the PSUM rows
                   (x and x*x rows overlapped with the tail of the y stream).
Partials land in engine-local SBUF tiles, DMA'd out as three tensors.
The first and last samples' x/y stream in halves to shrink pipeline fill
and tail. Partition reduction and final scalar math run on host in f64.

The stock TileContext epilogue (drain -> barrier -> gpsimd dma_reset +
sem_clear -> barrier) is trimmed (no dma_reset, no second barrier); sems are
still cleared so the NEFF re-executes correctly (verified deterministic
across repeated calls). A bare TileContext kernel measures ~11us of fixed
head+tail overhead (runtime preamble + EVSEM wind-down) that bounds what any
kernel shape can achieve here.
"""
import os
import sys

import numpy as np

for _p in ("/opt/trn_rl_repo", "/root/.axon_site/_ro/trn_rl_repo"):
    if os.path.isdir(_p) and _p not in sys.path:
        sys.path.append(_p)

import concourse.bass as bass
import concourse.mybir as mybir
import concourse.tile as tile
from concourse import bacc
from concourse.bass_utils import run_bass_kernel_spmd

NCORES = 8
B = 64
SPB = B // NCORES          # samples per core
P = 128                    # SBUF partitions
N = 1 * 480 * 640          # elements per sample
F = N // P                 # free dim per partition (2400)
H1 = F // 2                # first/last-sample split point (1200)
XD = 1275                  # sum(x*x) cols via DVE stt (rest: ACT Square)
EPS = 1e-8
NSEG = SPB + 2             # accum columns (first+last samples use two each)

FP32 = mybir.dt.float32
BF16 = mybir.dt.bfloat16

_CACHE = {}
LAST_RESULTS = None


class FastTileContext(tile.TileContext):
    """TileContext with a cheaper kernel-tail epilogue."""

    def _drain_and_barrier(self, tick_clock, wait_clock):
        if not os.environ.get("CCLOSS_FASTTAIL", "1") == "1":
            return super()._drain_and_barrier(tick_clock, wait_clock)
        nc = self.nc
        # Final drain waits for every sem lane's terminal value (covers all
        # DMA completions).  No barrier / sem_clear: the bass preamble
        # re-clears the kernel sem range at entry on every execution, so
        # end-state does not affect re-runs (verified deterministic).
        drain_inst = nc.sync.drain()
        wait_clock.add_sem_waits(
            drain_inst.ins, tile.ScopedClock({None: tick_clock.global_clock})
        )
        popped = nc._tile_sem_poison_stack.pop()
        assert popped is self._sem_poison
        sems = list(self.sems.allocated().values())
        sem_nums = [s.num if hasattr(s, "num") else s for s in sems]
        nc._state.prepend_free_semaphores(sem_nums)
        for poison_set in nc._tile_sem_poison_stack:
            poison_set.update(sem_nums)


def _chunks(lo, hi, step=480):
    out = []
    c = lo
    while c < hi:
        out.append((c, min(c + step, hi)))
        c = min(c + step, hi)
    return out


def _build():
    nc = bacc.Bacc("TRN2", target_bir_lowering=False, debug=False,
                   enable_asserts=False)
    yp_d = nc.dram_tensor("yp", (SPB, P, F), BF16, kind="ExternalInput").ap()
    yt_d = nc.dram_tensor("yt", (SPB, P, F), BF16, kind="ExternalInput").ap()
    # per-partition partials:
    #   dve: [P, 2*NSEG] = sum(x*y) segs 0.., sum(x*x)[stt part] segs NSEG..
    #   act: [P, 2*NSEG] = sum(y^2) segs 0.., sum(x*x)[sq part] segs NSEG..
    #   pe:  [SPB, 2]    = fully-reduced sum(x), sum(y)
    dve_d = nc.dram_tensor("dve", (P, 2 * NSEG), FP32,
                           kind="ExternalOutput").ap()
    act_d = nc.dram_tensor("act", (P, 2 * NSEG), FP32,
                           kind="ExternalOutput").ap()
    pe_d = nc.dram_tensor("pe", (SPB, 2), FP32, kind="ExternalOutput").ap()

    with FastTileContext(nc) as tc:
        with (
            tc.tile_pool(name="data", bufs=10) as data,
            tc.tile_pool(name="jdve", bufs=2) as jdve,
            tc.tile_pool(name="jact", bufs=2) as jact,
            tc.tile_pool(name="persist", bufs=1) as persist,
            tc.tile_pool(name="psum", bufs=1, space="PSUM") as psum,
        ):
            st_dve = persist.tile([P, 2 * NSEG], FP32)
            st_act = persist.tile([P, 2 * NSEG], FP32)
            st_pe = persist.tile([SPB, 2], FP32)
            # one-hot stationary source: ones16[:, SPB] == 1, rest 0;
            # sample s's stationary is the sliding view ones16[:, SPB-s:2*SPB-s]
            ones16 = persist.tile([P, 2 * SPB], BF16)
            nc.gpsimd.memset(ones16[:], 0.0)
            nc.gpsimd.memset(ones16[:, SPB:SPB + 1], 1.0)

            ps_x = psum.tile([SPB, 480], FP32)
            ps_y = psum.tile([SPB, 480], FP32)

            nseg = {"x": SPB + 2, "y": SPB + 2}
            mm_seen = {"x": False, "y": False}
            mm_done = {"x": 0, "y": 0}

            def pe_sums(ps, which, xt, s, w):
                """Accumulate per-column sums of xt[:, 0:w] into PSUM row s."""
                stat = ones16[:, SPB - s:2 * SPB - s]
                mm_done[which] += 1
                last_group = mm_done[which] == nseg[which]
                cks = _chunks(0, w)
                for i, (clo, chi) in enumerate(cks):
                    start = not mm_seen[which]
                    mm_seen[which] = True
                    stop = last_group and i == len(cks) - 1
                    nc.tensor.matmul(
                        ps[:, 0:chi - clo], stat, xt[:, clo:chi],
                        start=start, stop=stop, skip_group_check=True,
                    )

            def dve_sum(out_col, in0, in1, cols):
                prod = jdve.tile([P, cols], BF16, tag="jdve", name="jd")
                nc.vector.scalar_tensor_tensor(
                    out=prod[:], in0=in0, scalar=1.0, in1=in1,
                    op0=mybir.AluOpType.mult, op1=mybir.AluOpType.mult,
                    accum_out=st_dve[:, out_col:out_col + 1],
                )

            def act_sq(out_col, part, cols):
                sq = jact.tile([P, cols], BF16, tag="jact", name="ja")
                nc.scalar.activation(
                    sq[:], part, mybir.ActivationFunctionType.Square,
                    accum_out=st_act[:, out_col:out_col + 1],
                )

            def extract(ps, col):
                j = jact.tile([SPB, 480], FP32, tag="jpe", bufs=2, name="je")
                nc.scalar.activation(
                    j[:], ps[:], mybir.ActivationFunctionType.Copy,
                    accum_out=st_pe[:, col:col + 1],
                )

            def x_ops(s, seg, xt, w, frac):
                """x-side compute for one x tile of width w (frac of sample)."""
                xd = int(XD * frac)
                dve_sum(NSEG + seg, xt[:, 0:xd], xt[:, 0:xd], xd)
                act_sq(NSEG + seg, xt[:, xd:w], w - xd)
                pe_sums(ps_x, "x", xt, s, w)

            def y_ops(s, seg, xt, yt, w):
                """y-side compute for matching x/y tiles of width w."""
                dve_sum(seg, xt[:], yt[:], w)
                act_sq(seg, yt[:], w)
                pe_sums(ps_y, "y", yt, s, w)

            # segment table: (sample, seg_col, col_lo, col_hi)
            segs = []
            seg = 0
            for s in range(SPB):
                if s in (0, SPB - 1):
                    segs.append((s, seg, 0, H1)); seg += 1
                    segs.append((s, seg, H1, F)); seg += 1
                else:
                    segs.append((s, seg, 0, F)); seg += 1
            assert seg == NSEG

            for s, sg, lo, hi in segs:
                xt = data.tile([P, hi - lo], BF16, tag="xd", bufs=5, name="xt")
                nc.sync.dma_start(xt[:], yp_d[s, :, lo:hi])
                yt = data.tile([P, hi - lo], BF16, tag="yd", bufs=5, name="yt")
                nc.scalar.dma_start(yt[:], yt_d[s, :, lo:hi])
                w = hi - lo
                frac = w / F
                x_ops(s, sg, xt, w, frac)
                if sg == NSEG - 1:
                    # x PSUM reduce overlaps the tail of the y stream
                    extract(ps_x, 0)
                y_ops(s, sg, xt, yt, w)
            extract(ps_y, 1)

            nc.sync.dma_start(dve_d[:], st_dve[:])
            nc.scalar.dma_start(act_d[:], st_act[:])
            nc.sync.dma_start(pe_d[:], st_pe[:])

    nc.compile()
    return nc


def _get_nc():
    if "nc" not in _CACHE:
        _CACHE["nc"] = _build()
    return _CACHE["nc"]


def _to_bf16(a):
    import ml_dtypes
    return np.ascontiguousarray(
        np.asarray(a, dtype=np.float32).reshape(B, P, F)
    ).astype(ml_dtypes.bfloat16)


def kernel(y_pred: np.ndarray, y_true: np.ndarray) -> np.ndarray:
    global LAST_RESULTS
    nc = _get_nc()

    yp = _to_bf16(y_pred)
    yt = _to_bf16(y_true)

    in_maps = [
        {"yp": yp[c * SPB:(c + 1) * SPB], "yt": yt[c * SPB:(c + 1) * SPB]}
        for c in range(NCORES)
    ]
    trace = bool(os.environ.get("CCLOSS_TRACE"))
    try:
        res = run_bass_kernel_spmd(nc, in_maps, core_ids=list(range(NCORES)),
                                   trace=trace)
    except Exception:
        if not trace:
            raise
        res = run_bass_kernel_spmd(nc, in_maps, core_ids=list(range(NCORES)),
                                   trace=False)
    LAST_RESULTS = res

    # seg columns per sample
    seg_cols = {}
    seg = 0
    for s in range(SPB):
        nsg = 2 if s in (0, SPB - 1) else 1
        seg_cols[s] = list(range(seg, seg + nsg))
        seg += nsg

    r_all = np.empty(B, dtype=np.float64)
    n = float(N)
    for c in range(NCORES):
        dv = res.results[c]["dve"].astype(np.float64)   # [P, 2*NSEG]
        ac = res.results[c]["act"].astype(np.float64)   # [P, 2*NSEG]
        pe = res.results[c]["pe"].astype(np.float64)    # [SPB, 2]
        for s in range(SPB):
            cols = seg_cols[s]
            Sxy = sum(dv[:, t].sum() for t in cols)
            Sxx = sum(dv[:, NSEG + t].sum() + ac[:, NSEG + t].sum()
                      for t in cols)
            Syy = sum(ac[:, t].sum() for t in cols)
            Sx = pe[s, 0]
            Sy = pe[s, 1]

            cxx = Sxx - Sx * Sx / n            # sum((x-mu_x)^2)
            cyy = Syy - Sy * Sy / n
            cxy = Sxy - Sx * Sy / n
            sdx = np.sqrt(cxx / (n - 1.0)) + EPS
            sdy = np.sqrt(cyy / (n - 1.0)) + EPS

            num = cxy / (sdx * sdy)            # sum(a*b)
            saa = cxx / (sdx * sdx)            # sum(a*a)
            sbb = cyy / (sdy * sdy)
            r = num / np.sqrt(saa * sbb + EPS)
            r_all[c * SPB + s] = r

    loss = -r_all.mean()
    return np.array(loss, dtype=np.float32)


# revision 14
# speedup vs baseline: 1.0811x; 1.0811x over previous
"""CCLoss (Pearson correlation loss) Trainium2 kernel, 8-way data parallel.

Problem: y_pred ~ (64,1,480,640) f32, y_true ~ (64,1,480,640) f32.
reference: per-sample z-score (ddof=1) over (1,480,640), r = corr-like ratio,
loss = -mean(r).

Strategy: shard batch (64) across 8 cores, 8 samples/core. Inputs are
converted to bf16 on the host (quantization perturbs the loss by ~1e-3
relative, far under the 2e-2 gate) which halves HBM traffic; the kernel is
memory-bound (~9.8MB/core at ~420GB/s = 23.5us stream floor), but the
per-sample moment reductions make compute the slightly-longer pole
(~3.7us/sample across three engines vs 2.93us of DMA).

Five per-sample sums, one pass over the data, balanced across engines
(measured bf16 rates: DVE stt 1.07ns/col, DVE tensor_tensor 0.55, ACT square
0.91 + 185ns/accum-read, PE colsum-matmul ~0.73 incl ldweights):
  - VectorE (DVE): sum(x*y) full via scalar_tensor_tensor accum;
                   sum(x*x) cols [XP:XP+XD) via stt; x*x product tile for
                   cols [0:XP) via 2x tensor_tensor (PE reduces it)
  - ScalarE (ACT): sum(y^2) full + sum(x*x) cols [XP+XD:F) via Square accum
  - TensorE (PE):  sum(x), sum(y), and colsums of the x*x product tile:
                   ones-one-hot-stationary matmuls (sample s uses a [128,8]
                   stationary all-ones in column s) accumulate per-sample-row
                   column sums in PSUM; ACT copy+accum reduces the PSUM rows
                   (x and x*x rows overlapped with the tail of the y stream).
Partials land in engine-local SBUF tiles, DMA'd out as three tensors.
The first and last samples' x/y stream in halves to shrink pipeline fill
and tail. Partition reduction and final scalar math run on host in f64.

The stock TileContext epilogue (drain -> barrier -> gpsimd dma_reset +
sem_clear -> barrier) is trimmed (no dma_reset, no second barrier); sems are
still cleared so the NEFF re-executes correctly (verified deterministic
across repeated calls). A bare TileContext kernel measures ~11us of fixed
head+tail overhead (runtime preamble + EVSEM wind-down) that bounds what any
kernel shape can achieve here.
"""
import os
import sys

import numpy as np

for _p in ("/opt/trn_rl_repo", "/root/.axon_site/_ro/trn_rl_repo"):
    if os.path.isdir(_p) and _p not in sys.path:
        sys.path.append(_p)

import concourse.bass as bass
import concourse.mybir as mybir
import concourse.tile as tile
from concourse import bacc
from concourse.bass_utils import run_bass_kernel_spmd

NCORES = 8
B = 64
SPB = B // NCORES          # samples per core
P = 128                    # SBUF partitions
N = 1 * 480 * 640          # elements per sample
F = N // P                 # free dim per partition (2400)
H1 = F // 2                # first/last-sample split point (1200)
XD = 1275                  # sum(x*x) cols via DVE stt (rest: ACT Square)
EPS = 1e-8
NSEG = SPB + 2             # accum columns (first+last samples use two each)

FP32 = mybir.dt.float32
BF16 = mybir.dt.bfloat16

_CACHE = {}
LAST_RESULTS = None


class FastTileContext(tile.TileContext):
    """TileContext with a cheaper kernel-tail epilogue."""

    def _drain_and_barrier(self, tick_clock, wait_clock):
        if not os.environ.get("CCLOSS_FASTTAIL", "1") == "1":
            return super()._drain_and_barrier(tick_clock, wait_clock)
        nc = self.nc
        # Final drain waits for every sem lane's terminal value (covers all
        # DMA completions).  No barrier / sem_clear: the bass preamble
        # re-clears the kernel sem range at entry on every execution, so
        # end-state does not affect re-runs (verified deterministic).
        drain_inst = nc.sync.drain()
        wait_clock.add_sem_waits(
            drain_inst.ins, tile.ScopedClock({None: tick_clock.global_clock})
        )
        popped = nc._tile_sem_poison_stack.pop()
        assert popped is self._sem_poison
        sems = list(self.sems.allocated().values())
        sem_nums = [s.num if hasattr(s, "num") else s for s in sems]
        nc._state.prepend_free_semaphores(sem_nums)
        for poison_set in nc._tile_sem_poison_stack:
            poison_set.update(sem_nums)


def _chunks(lo, hi, step=480):
    out = []
    c = lo
    while c < hi:
        out.append((c, min(c + step, hi)))
        c = min(c + step, hi)
    return out


def _build():
    nc = bacc.Bacc("TRN2", target_bir_lowering=False, debug=False,
                   enable_asserts=False)
    yp_d = nc.dram_tensor("yp", (SPB, P, F), BF16, kind="ExternalInput").ap()
    yt_d = nc.dram_tensor("yt", (SPB, P, F), BF16, kind="ExternalInput").ap()
    # per-partition partials:
    #   dve: [P, 2*NSEG] = sum(x*y) segs 0.., sum(x*x)[stt part] segs NSEG..
    #   act: [P, 2*NSEG] = sum(y^2) segs 0.., sum(x*x)[sq part] segs NSEG..
    #   pe:  [SPB, 2]    = fully-reduced sum(x), sum(y)
    dve_d = nc.dram_tensor("dve", (P, 2 * NSEG), FP32,
                           kind="ExternalOutput").ap()
    act_d = nc.dram_tensor("act", (P, 2 * NSEG), FP32,
                           kind="ExternalOutput").ap()
    pe_d = nc.dram_tensor("pe", (SPB, 2), FP32, kind="ExternalOutput").ap()

    with FastTileContext(nc) as tc:
        with (
            tc.tile_pool(name="data", bufs=10) as data,
            tc.tile_pool(name="jdve", bufs=2) as jdve,
            tc.tile_pool(name="jact", bufs=2) as jact,
            tc.tile_pool(name="persist", bufs=1) as persist,
            tc.tile_pool(name="psum", bufs=1, space="PSUM") as psum,
        ):
            st_dve = persist.tile([P, 2 * NSEG], FP32)
            st_act = persist.tile([P, 2 * NSEG], FP32)
            st_pe = persist.tile([SPB, 2], FP32)
            # one-hot stationary source: ones16[:, SPB] == 1, rest 0;
            # sample s's stationary is the sliding view ones16[:, SPB-s:2*SPB-s]
            ones16 = persist.tile([P, 2 * SPB], BF16)
            nc.gpsimd.memset(ones16[:], 0.0)
            nc.gpsimd.memset(ones16[:, SPB:SPB + 1], 1.0)

            ps_x = psum.tile([SPB, 480], FP32)
            ps_y = psum.tile([SPB, 480], FP32)

            nseg = {"x": SPB + 2, "y": SPB + 2}
            mm_seen = {"x": False, "y": False}
            mm_done = {"x": 0, "y": 0}

            def pe_sums(ps, which, xt, s, w):
                """Accumulate per-column sums of xt[:, 0:w] into PSUM row s."""
                stat = ones16[:, SPB - s:2 * SPB - s]
                mm_done[which] += 1
                last_group = mm_done[which] == nseg[which]
                cks = _chunks(0, w)
                for i, (clo, chi) in enumerate(cks):
                    start = not mm_seen[which]
                    mm_seen[which] = True
                    stop = last_group and i == len(cks) - 1
                    nc.tensor.matmul(
                        ps[:, 0:chi - clo], stat, xt[:, clo:chi],
                        start=start, stop=stop, skip_group_check=True,
                    )

            def dve_sum(out_col, in0, in1, cols):
                prod = jdve.tile([P, cols], BF16, tag="jdve", name="jd")
                nc.vector.scalar_tensor_tensor(
                    out=prod[:], in0=in0, scalar=1.0, in1=in1,
                    op0=mybir.AluOpType.mult, op1=mybir.AluOpType.mult,
                    accum_out=st_dve[:, out_col:out_col + 1],
                )

            def act_sq(out_col, part, cols):
                sq = jact.tile([P, cols], BF16, tag="jact", name="ja")
                nc.scalar.activation(
                    sq[:], part, mybir.ActivationFunctionType.Square,
                    accum_out=st_act[:, out_col:out_col + 1],
                )

            def extract(ps, col):
                j = jact.tile([SPB, 480], FP32, tag="jpe", bufs=2, name="je")
                nc.scalar.activation(
                    j[:], ps[:], mybir.ActivationFunctionType.Copy,
                    accum_out=st_pe[:, col:col + 1],
                )

            def x_ops(s, seg, xt, w, frac):
                """x-side compute for one x tile of width w (frac of sample)."""
                xd = int(XD * frac)
                dve_sum(NSEG + seg, xt[:, 0:xd], xt[:, 0:xd], xd)
                act_sq(NSEG + seg, xt[:, xd:w], w - xd)
                pe_sums(ps_x, "x", xt, s, w)

            def y_ops(s, seg, xt, yt, w):
                """y-side compute for matching x/y tiles of width w."""
                dve_sum(seg, xt[:], yt[:], w)
                act_sq(seg, yt[:], w)
                pe_sums(ps_y, "y", yt, s, w)

            # segment table: (sample, seg_col, col_lo, col_hi)
            segs = []
            seg = 0
            for s in range(SPB):
                if s in (0, SPB - 1):
                    segs.append((s, seg, 0, H1)); seg += 1
                    segs.append((s, seg, H1, F)); seg += 1
                else:
                    segs.append((s, seg, 0, F)); seg += 1
            assert seg == NSEG

            for s, sg, lo, hi in segs:
                xt = data.tile([P, hi - lo], BF16, tag="xd", bufs=5, name="xt")
                nc.sync.dma_start(xt[:], yp_d[s, :, lo:hi])
                yt = data.tile([P, hi - lo], BF16, tag="yd", bufs=5, name="yt")
                nc.sync.dma_start(yt[:], yt_d[s, :, lo:hi])
                w = hi - lo
                frac = w / F
                x_ops(s, sg, xt, w, frac)
                if sg == NSEG - 1:
                    # x PSUM reduce overlaps the tail of the y stream
                    extract(ps_x, 0)
                y_ops(s, sg, xt, yt, w)
            extract(ps_y, 1)

            nc.sync.dma_start(dve_d[:], st_dve[:])
            nc.scalar.dma_start(act_d[:], st_act[:])
            nc.sync.dma_start(pe_d[:], st_pe[:])

    nc.compile()
    return nc


def _get_nc():
    if "nc" not in _CACHE:
        _CACHE["nc"] = _build()
    return _CACHE["nc"]


def _to_bf16(a):
    import ml_dtypes
    return np.ascontiguousarray(
        np.asarray(a, dtype=np.float32).reshape(B, P, F)
    ).astype(ml_dtypes.bfloat16)


def kernel(y_pred: np.ndarray, y_true: np.ndarray) -> np.ndarray:
    global LAST_RESULTS
    nc = _get_nc()

    yp = _to_bf16(y_pred)
    yt = _to_bf16(y_true)

    in_maps = [
        {"yp": yp[c * SPB:(c + 1) * SPB], "yt": yt[c * SPB:(c + 1) * SPB]}
        for c in range(NCORES)
    ]
    trace = bool(os.environ.get("CCLOSS_TRACE"))
    try:
        res = run_bass_kernel_spmd(nc, in_maps, core_ids=list(range(NCORES)),
                                   trace=trace)
    except Exception:
        if not trace:
            raise
        res = run_bass_kernel_spmd(nc, in_maps, core_ids=list(range(NCORES)),
                                   trace=False)
    LAST_RESULTS = res

    # seg columns per sample
    seg_cols = {}
    seg = 0
    for s in range(SPB):
        nsg = 2 if s in (0, SPB - 1) else 1
        seg_cols[s] = list(range(seg, seg + nsg))
        seg += nsg

    r_all = np.empty(B, dtype=np.float64)
    n = float(N)
    for c in range(NCORES):
        dv = res.results[c]["dve"].astype(np.float64)   # [P, 2*NSEG]
        ac = res.results[c]["act"].astype(np.float64)   # [P, 2*NSEG]
        pe = res.results[c]["pe"].astype(np.float64)    # [SPB, 2]
        for s in range(SPB):
            cols = seg_cols[s]
            Sxy = sum(dv[:, t].sum() for t in cols)
            Sxx = sum(dv[:, NSEG + t].sum() + ac[:, NSEG + t].sum()
                      for t in cols)
            Syy = sum(ac[:, t].sum() for t in cols)
            Sx = pe[s, 0]
            Sy = pe[s, 1]

            cxx = Sxx - Sx * Sx / n            # sum((x-mu_x)^2)
            cyy = Syy - Sy * Sy / n
            cxy = Sxy - Sx * Sy / n
            sdx = np.sqrt(cxx / (n - 1.0)) + EPS
            sdy = np.sqrt(cyy / (n - 1.0)) + EPS

            num = cxy / (sdx * sdy)            # sum(a*b)
            saa = cxx / (sdx * sdx)            # sum(a*a)
            sbb = cyy / (sdy * sdy)
            r = num / np.sqrt(saa * sbb + EPS)
            r_all[c * SPB + s] = r

    loss = -r_all.mean()
    return np.array(loss, dtype=np.float32)
